# revision 1
# baseline (speedup 1.0000x reference)
"""Trainium2 Bass kernel for nn_AttentionModel (B=4, S=4096, E=2048) on 8 cores.

Sharding: data-parallel over batch B (4) x tensor-parallel over the E output
dim of the Q projection (2). Core c handles batch b=c//2 and scores rows
e in [h*1024, (h+1)*1024) with h=c%2. Each core computes k, v in full for its
batch (duplicated within the pair; avoids collectives), q for its half, then
scores -> softmax -> attn @ v for its half of the output rows.

All GEMMs run on the PE array in float32r (full-rate fp32, ~1e-4 rel err).
Layouts are chosen so every matmul contracts over the partition dim:
  qT,kT [s, e]: stationary = transposed-x column tiles (host provides x^T)
  v     [f, s]: stationary = Wv^T column tiles, moving = x^T rows
  scores[e, f] = qT.T @ kT contracting s; softmax over free dim f
  outT  [s, e] = v.T @ attnT contracting f (host transposes back)
Q/K biases enter via rank-1 (K=1) matmul accumulation; V bias via the
per-partition bias of the activation-copy eviction. The 1/sqrt(E) score scale
is folded into Wq/bq on the host.
"""

import sys

sys.path.insert(0, "/opt/trn_rl_repo")

from contextlib import ExitStack

import numpy as np

import concourse.bass as bass
import concourse.mybir as mybir
import concourse.tile as tile
from concourse import bacc
from concourse.bass_utils import run_bass_kernel_spmd
from concourse.masks import make_identity

f32 = mybir.dt.float32
f32r = mybir.dt.float32r

B, S, E = 4, 4096, 2048
EH = E // 2          # per-core q rows (embed half)
N = 512              # moving free-dim per matmul (one PSUM bank)
SKT = S // 128       # 32 s k-tiles
EKT = E // 128       # 16 e k-tiles
N_CORES = 8


def build_kernel():
    nc = bacc.Bacc("TRN2", debug=False, target_bir_lowering=False)

    xt = nc.dram_tensor("xt", [E, S], f32r, kind="ExternalInput")        # x^T
    xtt = nc.dram_tensor("xtt", [SKT, 128, EKT, 128], f32r, kind="ExternalInput")  # x^T tiled [st,e,kt,s]
    wqk = nc.dram_tensor("wqk", [E, E + EH], f32r, kind="ExternalInput")  # [Wk^T | Wq_h^T/sqrt(E)]
    bkq = nc.dram_tensor("bkq", [1, E + EH], f32r, kind="ExternalInput")  # [bk | bq_h/sqrt(E)]
    wv = nc.dram_tensor("wv", [EKT, E, 128], f32r, kind="ExternalInput")  # Wv^T tiled by f
    bv = nc.dram_tensor("bv", [128, EKT], f32, kind="ExternalInput")      # bv packed per f-tile
    ones_d = nc.dram_tensor("ones", [1, 128], f32r, kind="ExternalInput")
    outt = nc.dram_tensor("outt", [EH, S], f32, kind="ExternalOutput")

    with tile.TileContext(nc) as tc, ExitStack() as ctx:
        dram = ctx.enter_context(tc.tile_pool(name="dram", bufs=1, space="DRAM"))
        qt_d = dram.tile([EH // 128, 128, SKT, 128], f32r)
        kt_d = dram.tile([S, E], f32r)
        v_d = dram.tile([E, S], f32r)
        sc_d = dram.tile([EH, E], f32)

        const = ctx.enter_context(tc.tile_pool(name="const", bufs=1))
        ones_sb = const.tile([1, 128], f32r)
        nc.sync.dma_start(ones_sb[:, :], ones_d[:, :])
        ident = const.tile([128, 128], f32)
        make_identity(nc, ident[:, :])
        bv_sb = const.tile([128, EKT], f32)
        nc.sync.dma_start(bv_sb[:, :], bv[:, :])
        bkq_sb = const.tile([1, E + EH], f32r)
        nc.sync.dma_start(bkq_sb[:, :], bkq[:, :])

        # ---- Phase 1ab: qT [s, e_h] and kT [s, f] in two f-passes ----
        # pass 0: k cols [0:1024) + q cols (wqk cols [0:1024) and [2048:3072))
        # pass 1: k cols [1024:2048) (wqk cols [1024:2048))
        for p1pass in range(2):
            w_cols = (
                [(0, 1024), (E, E + EH)] if p1pass == 0 else [(1024, 2048)]
            )
            w_width = sum(b - a for a, b in w_cols)
            with (
                tc.tile_pool(name=f"p1_w{p1pass}", bufs=1) as p_w,
                tc.tile_pool(name=f"p1_xc{p1pass}", bufs=3) as p_xc,
                tc.tile_pool(name=f"p1_st{p1pass}", bufs=2) as p_st,
                tc.tile_pool(name=f"p1_ps{p1pass}", bufs=2, space="PSUM") as p_ps,
            ):
                w_sb = p_w.tile([128, EKT, w_width], f32r)
                bias_sb = p_w.tile([1, w_width], f32r)
                off = 0
                for a, b_ in w_cols:
                    nc.sync.dma_start(bias_sb[:, off:off + (b_ - a)], bkq[:, a:b_])
                    off += b_ - a
                for ekt in range(EKT):
                    off = 0
                    for a, b_ in w_cols:
                        nc.sync.dma_start(
                            w_sb[:, ekt, off:off + (b_ - a)],
                            wqk[ekt * 128:(ekt + 1) * 128, a:b_],
                        )
                        off += b_ - a
                nchunks = w_width // N
                for st in range(SKT):
                    xtc = p_xc.tile([128, EKT, 128], f32r, tag="xtc")
                    nc.scalar.dma_start(xtc[:, :, :], xtt[st])
                    ps = p_ps.tile([128, w_width], f32, tag="ps")
                    for ekt in range(EKT):
                        lhsT = xtc[:, ekt, :]
                        for fc in range(nchunks):
                            nc.tensor.matmul(
                                ps[:, fc * N:(fc + 1) * N],
                                lhsT,
                                w_sb[:, ekt, fc * N:(fc + 1) * N],
                                start=(ekt == 0),
                                stop=False,
                            )
                    for fc in range(nchunks):
                        nc.tensor.matmul(
                            ps[:, fc * N:(fc + 1) * N],
                            ones_sb[:, :],
                            bias_sb[:, fc * N:(fc + 1) * N],
                            start=False,
                            stop=True,
                        )
                    rows = slice(st * 128, (st + 1) * 128)
                    if p1pass == 0:
                        ksb = p_st.tile([128, 1024], f32r, tag="ksb")
                        nc.scalar.copy(ksb[:, :], ps[:, 0:1024])
                        nc.sync.dma_start(kt_d[rows, 0:1024], ksb[:, :])
                        qsb = p_st.tile([128, EH], f32r, tag="qsb")
                        nc.scalar.copy(qsb[:, :], ps[:, 1024:2048])
                        nc.sync.dma_start(
                            qt_d[:, :, st, :].rearrange("et p e -> p et e"),
                            qsb[:, :].rearrange("p (et e) -> p et e", e=128),
                        )
                    else:
                        ksb = p_st.tile([128, 1024], f32r, tag="ksb")
                        nc.scalar.copy(ksb[:, :], ps[:, 0:1024])
                        nc.sync.dma_start(kt_d[rows, 1024:2048], ksb[:, :])

        # ---- Phase 1c: v [f, s] ----
        with (
            tc.tile_pool(name="p1c_x", bufs=1) as p_xh,
            tc.tile_pool(name="p1c_w", bufs=3) as p_wv,
            tc.tile_pool(name="p1c_st", bufs=3) as p_vst,
            tc.tile_pool(name="p1c_ps", bufs=2, space="PSUM") as p_psv,
        ):
            for sh in range(2):
                xth = p_xh.tile([128, EKT, S // 2], f32r, tag="xth")
                for ekt in range(EKT):
                    nc.sync.dma_start(
                        xth[:, ekt, :],
                        xt[ekt * 128:(ekt + 1) * 128,
                           sh * (S // 2):(sh + 1) * (S // 2)],
                    )
                for ft in range(EKT):
                    wvc = p_wv.tile([128, EKT, 128], f32r, tag="wvc")
                    nc.scalar.dma_start(
                        wvc[:, :, :],
                        wv[ft].rearrange("(kt p) f -> p kt f", p=128),
                    )
                    psv = p_psv.tile([128, S // 2], f32, tag="psv")
                    for ekt in range(EKT):
                        for sc in range(4):
                            nc.tensor.matmul(
                                psv[:, sc * N:(sc + 1) * N],
                                wvc[:, ekt, :],
                                xth[:, ekt, sc * N:(sc + 1) * N],
                                start=(ekt == 0),
                                stop=(ekt == EKT - 1),
                            )
                    vsb = p_vst.tile([128, S // 2], f32r, tag="vsb")
                    nc.scalar.activation(
                        vsb[:, :], psv[:, :],
                        mybir.ActivationFunctionType.Identity,
                        bias=bv_sb[:, ft:ft + 1], scale=1.0,
                    )
                    nc.sync.dma_start(
                        v_d[ft * 128:(ft + 1) * 128,
                            sh * (S // 2):(sh + 1) * (S // 2)],
                        vsb[:, :],
                    )

        # ---- Phase 2: scores [e_h, f] = qT.T @ kT ----
        with (
            tc.tile_pool(name="p2_k", bufs=1) as p_kh,
            tc.tile_pool(name="p2_q", bufs=2) as p_qc,
            tc.tile_pool(name="p2_st", bufs=3) as p_sst,
            tc.tile_pool(name="p2_ps", bufs=2, space="PSUM") as p_ps2,
        ):
            for fh in range(2):
                kth = p_kh.tile([128, SKT, E // 2], f32r, tag="kth")
                for skt in range(SKT):
                    nc.sync.dma_start(
                        kth[:, skt, :],
                        kt_d[skt * 128:(skt + 1) * 128,
                             fh * (E // 2):(fh + 1) * (E // 2)],
                    )
                for et in range(EH // 128):
                    qtc = p_qc.tile([128, SKT, 128], f32r, tag="qtc")
                    nc.scalar.dma_start(qtc[:, :, :], qt_d[et])
                    ps2 = p_ps2.tile([128, E // 2], f32, tag="ps2")
                    for skt in range(SKT):
                        for fc in range(2):
                            nc.tensor.matmul(
                                ps2[:, fc * N:(fc + 1) * N],
                                qtc[:, skt, :],
                                kth[:, skt, fc * N:(fc + 1) * N],
                                start=(skt == 0),
                                stop=(skt == SKT - 1),
                            )
                    ssb = p_sst.tile([128, E // 2], f32, tag="ssb")
                    nc.scalar.copy(ssb[:, :], ps2[:, :])
                    nc.sync.dma_start(
                        sc_d[et * 128:(et + 1) * 128,
                             fh * (E // 2):(fh + 1) * (E // 2)],
                        ssb[:, :],
                    )

        # ---- Phase 3 + 4: softmax, attn^T, outT = v.T @ attnT ----
        with (
            tc.tile_pool(name="p3_at", bufs=1) as p_at,
            tc.tile_pool(name="p3_sm", bufs=2) as p_sm,
            tc.tile_pool(name="p3_ps", bufs=2, space="PSUM") as p_pst,
        ):
            attnT = p_at.tile([128, EKT, EH], f32r)
            for et in range(EH // 128):
                scs = p_sm.tile([128, E], f32, tag="scs")
                nc.scalar.dma_start(scs[:, :], sc_d[et * 128:(et + 1) * 128, :])
                negmax = p_sm.tile([128, 1], f32, tag="negmax")
                nc.vector.tensor_reduce(
                    out=negmax[:, :], in_=scs[:, :], op=mybir.AluOpType.max,
                    axis=mybir.AxisListType.X, negate=True,
                )
                attn = p_sm.tile([128, E], f32, tag="attn")
                sums = p_sm.tile([128, 1], f32, tag="sums")
                nc.scalar.activation(
                    attn[:, :], scs[:, :], mybir.ActivationFunctionType.Exp,
                    bias=negmax[:, 0:1], scale=1.0, accum_out=sums[:, 0:1],
                )
                rsum = p_sm.tile([128, 1], f32, tag="rsum")
                nc.vector.reciprocal(rsum[:, :], sums[:, :])
                attn2 = p_sm.tile([128, E], f32, tag="attn2")
                nc.vector.tensor_scalar_mul(attn2[:, :], attn[:, :], rsum[:, 0:1])
                for half in range(2):
                    pst = p_pst.tile([128, 1024], f32, tag="pst")
                    for c in range(8):
                        fkt = half * 8 + c
                        nc.tensor.transpose(
                            pst[:, c * 128:(c + 1) * 128],
                            attn2[:, fkt * 128:(fkt + 1) * 128],
                            ident[:, :],
                        )
                    nc.vector.tensor_copy(
                        attnT[:, half * 8:(half + 1) * 8,
                              et * 128:(et + 1) * 128],
                        pst[:, :].rearrange("p (c f) -> p c f", f=128),
                    )

            with (
                tc.tile_pool(name="p4_v", bufs=1) as p_vb,
                tc.tile_pool(name="p4_st", bufs=3) as p_ost,
                tc.tile_pool(name="p4_ps", bufs=2, space="PSUM") as p_ps4,
            ):
                SB = 1024
                for sb in range(S // SB):
                    vb = p_vb.tile([128, EKT, SB], f32r, tag="vb")
                    for fkt in range(EKT):
                        nc.scalar.dma_start(
                            vb[:, fkt, :],
                            v_d[fkt * 128:(fkt + 1) * 128,
                                sb * SB:(sb + 1) * SB],
                        )
                    for et in range(EH // 128):
                        ps4 = p_ps4.tile([128, SB], f32, tag="ps4")
                        for fkt in range(EKT):
                            for sc in range(SB // N):
                                nc.tensor.matmul(
                                    ps4[:, sc * N:(sc + 1) * N],
                                    attnT[:, fkt, et * 128:(et + 1) * 128],
                                    vb[:, fkt, sc * N:(sc + 1) * N],
                                    start=(fkt == 0),
                                    stop=(fkt == EKT - 1),
                                )
                        osb = p_ost.tile([128, SB], f32, tag="osb")
                        nc.scalar.copy(osb[:, :], ps4[:, :])
                        nc.sync.dma_start(
                            outt[et * 128:(et + 1) * 128,
                                 sb * SB:(sb + 1) * SB],
                            osb[:, :],
                        )

    nc.compile()
    return nc


_NC_CACHE = {}


def _get_nc():
    if "nc" not in _NC_CACHE:
        _NC_CACHE["nc"] = build_kernel()
    return _NC_CACHE["nc"]


def make_in_maps(x, Wq, bq, Wk, bk, Wv, bv):
    sc = np.float32(1.0 / np.sqrt(E))
    in_maps = []
    wk_t = np.ascontiguousarray(Wk.T)                       # [E, E]
    wv_t = np.ascontiguousarray(Wv.T)                       # [E, E]
    wv_tiled = np.ascontiguousarray(
        wv_t.reshape(E, EKT, 128).transpose(1, 0, 2)        # [EKT, E, 128]
    )
    bv_packed = np.ascontiguousarray(bv.reshape(EKT, 128).T)  # [128, EKT]
    for c in range(N_CORES):
        b, h = c // 2, c % 2
        xt = np.ascontiguousarray(x[b].T)                   # [E, S]
        xtt = np.ascontiguousarray(
            x[b].reshape(SKT, 128, EKT, 128).transpose(0, 3, 2, 1)
        )                                                   # [st, e, kt, s]
        wq_h = Wq[h * EH:(h + 1) * EH, :] * sc              # [EH, E]
        wqk = np.ascontiguousarray(
            np.concatenate([wk_t, wq_h.T], axis=1)          # [E, E+EH]
        )
        bkq = np.concatenate([bk, bq[h * EH:(h + 1) * EH] * sc])[None, :]
        in_maps.append({
            "xt": xt,
            "xtt": xtt,
            "wqk": wqk,
            "bkq": np.ascontiguousarray(bkq.astype(np.float32)),
            "wv": wv_tiled,
            "bv": bv_packed,
            "ones": np.ones((1, 128), np.float32),
        })
    return in_maps


def run(in_maps, trace=False, **kwargs):
    nc = _get_nc()
    return run_bass_kernel_spmd(
        nc, in_maps, core_ids=list(range(N_CORES)), trace=trace, **kwargs
    )


def kernel(x, Wq, bq, Wk, bk, Wv, bv):
    x = np.asarray(x, dtype=np.float32)
    in_maps = make_in_maps(
        x,
        np.asarray(Wq, np.float32), np.asarray(bq, np.float32),
        np.asarray(Wk, np.float32), np.asarray(bk, np.float32),
        np.asarray(Wv, np.float32), np.asarray(bv, np.float32),
    )
    res = run(in_maps, trace=False)
    out = np.empty((B, E, S), dtype=np.float32)
    for c in range(N_CORES):
        b, h = c // 2, c % 2
        out[b, h * EH:(h + 1) * EH, :] = res.results[c]["outt"]
    return out



# revision 12
# speedup vs baseline: 1.5503x; 1.5503x over previous
"""Trainium2 Bass kernel for nn_AttentionModel (B=4, S=4096, E=2048) on 8 cores.

Sharding: data-parallel over batch B (4) x tensor-parallel over the E output
dim of the Q projection (2). Core c handles batch b=c//2 and scores rows
m in [h*1024, (h+1)*1024) with h=c%2. No collectives.

Two reassociations cut per-core FLOPs from 120G (qkv+scores+out) to 64.5G:

  scores = (Wq_h/sqrt(E)) @ G @ Wk^T + rank-2 bias term,
  with G = x^T x the symmetric Gram matrix: only 160/256 of its 128x128
  tiles are computed on the PE; the 96 mirror tiles are PE transposes.
  This replaces the q and k projections and the [m,S]x[S,f] score GEMM.

  out = attn @ v = (attn @ Wv) @ x^T + (attn.bv): the v projection is never
  materialized. P^T = Wv-tiles @ attn^T costs 8.6G vs v's 34.4G.

Softmax keeps exp() unnormalized through the P GEMM; 1/sum and the attn.bv
bias fold into the final eviction as per-partition scale/bias.

Bias correction (host-precomputed, r = sum_s x[s,:]):
  scores += btq (x) kr + (qr + S*btq) (x) bk,  btq = bq_h/sqrt(E),
  qr = (Wq_h/sqrt(E)) r, kr = Wk r  -- a single K=2 matmul accumulation.

All GEMMs run in float32r with moving free dim 512 (full rate).
"""

import sys

sys.path.insert(0, "/opt/trn_rl_repo")

from contextlib import ExitStack

import numpy as np

import concourse.bass as bass
import concourse.mybir as mybir
import concourse.tile as tile
from concourse import bacc
from concourse.bass_utils import run_bass_kernel_spmd
from concourse.masks import make_identity

f32 = mybir.dt.float32
f32r = mybir.dt.float32r

B, S, E = 4, 4096, 2048
EH = E // 2          # per-core scores rows (embed half)
EB = E // 128        # 16 e-blocks
MB = EH // 128       # 8 m-blocks
ST = S // 128        # 32 s-chunks
N_CORES = 8

Act = mybir.ActivationFunctionType
Alu = mybir.AluOpType
Ax = mybir.AxisListType


def build_kernel():
    nc = bacc.Bacc("TRN2", debug=False, target_bir_lowering=False)

    x_se = nc.dram_tensor("x_se", [S, E], f32r, kind="ExternalInput")   # x[b]
    xt = nc.dram_tensor("xt", [E, S], f32r, kind="ExternalInput")       # x[b]^T
    wqt = nc.dram_tensor("wqt", [E, EH], f32r, kind="ExternalInput")    # (Wq_h/sqrtE)^T
    wkt = nc.dram_tensor("wkt", [E, E], f32r, kind="ExternalInput")     # Wk^T
    wv_t = nc.dram_tensor("wv_t", [EB, EB, 128, 128], f32r, kind="ExternalInput")
    u_in = nc.dram_tensor("u_in", [2, EH], f32r, kind="ExternalInput")  # [btq; qr+S*btq]
    w_in = nc.dram_tensor("w_in", [2, E], f32r, kind="ExternalInput")   # [kr; bk]
    bv_in = nc.dram_tensor("bv_in", [128, EB, 256], f32r, kind="ExternalInput")
    outt = nc.dram_tensor("outt", [EH, S], f32, kind="ExternalOutput")

    with tile.TileContext(nc) as tc, ExitStack() as ctx:
        dram = ctx.enter_context(tc.tile_pool(name="dram", bufs=1, space="DRAM"))
        g_d = dram.tile([EB, EB, 128, 128], f32r)     # [fb][e] = G[e-blk, fb-blk]
        at_d = dram.tile([EB, MB, 128, 128], f32r)    # [e'b][mb] = A^T tile
        attn_d = dram.tile([MB, 128, E], f32r)        # unnormalized exp rows

        const = ctx.enter_context(tc.tile_pool(name="const", bufs=1))
        ident_f = const.tile([128, 128], f32)
        make_identity(nc, ident_f[:, :])
        ident = const.tile([128, 128], f32r)
        nc.scalar.copy(ident[:, :], ident_f[:, :])
        u_sb = const.tile([2, EH], f32r)
        nc.sync.dma_start(u_sb[:, :], u_in[:, :])
        w_sb = const.tile([2, E], f32r)
        nc.sync.dma_start(w_sb[:, :], w_in[:, :])
        bv_sb = const.tile([128, EB, 256], f32r)
        nc.sync.dma_start(bv_sb[:, :, :], bv_in[:, :, :])

        keep = ctx.enter_context(tc.tile_pool(name="keep", bufs=1))
        rsum_sb = keep.tile([128, MB], f32)      # 1/softmax-sum per m-block col
        abv_sb = keep.tile([128, MB], f32)       # attn@bv (unnormalized)
        biasf_sb = keep.tile([128, MB], f32)     # abv*rsum

        # ================= Phase G: symmetric Gram =================
        # direct regions (tile (row i, col j) of G):
        #   (a) i 0-3,  j 0-7    (b) i 4-7, j 4-7    (c) i 0-7, j 8-15
        #   (d) i 8-11, j 8-15   (e) i 12-15, j 12-15
        # mirrors (PE transpose): T(a:j4-7)->(4-7,0-3); T(c)->(8-15,0-7);
        #   T(d:j12-15)->(12-15,8-11)
        with tc.tile_pool(name="g_tstage", bufs=4) as p_tst:

            def evict(p_gst, ps, i, j0, jn, tag):
                # ps [128, jn*128] = G[i-block, j0:j0+jn) -> g_d[j][i]
                st_ = p_gst.tile([128, jn * 128], f32r, tag="gst",
                                 name=f"gst_{tag}")
                nc.scalar.copy(st_[:, :], ps[:, :])
                nc.sync.dma_start(
                    g_d[j0:j0 + jn, i].rearrange("j p e -> p j e"),
                    st_[:, :].rearrange("p (j e) -> p j e", e=128),
                )
                return st_

            def mirror(tp, st_, i, j0, js, tag):
                # write T(G[i, j]) -> g_d[i][j] for j in js (js contiguous)
                pst = tp.tile([128, 128 * len(js)], f32r, tag="tps")
                for t, j in enumerate(js):
                    nc.tensor.transpose(
                        pst[:, t * 128:(t + 1) * 128],
                        st_[:, (j - j0) * 128:(j - j0 + 1) * 128],
                        ident[:, :],
                    )
                ts_ = p_tst.tile([128, 128 * len(js)], f32r, tag="tst")
                nc.vector.tensor_copy(ts_[:, :], pst[:, :])
                nc.sync.dma_start(
                    g_d[i, js[0]:js[0] + len(js)].rearrange("j p e -> p j e"),
                    ts_[:, :].rearrange("p (j e) -> p j e", e=128),
                )

            with tc.tile_pool(name="g_xres", bufs=1) as p_res:
                xh = p_res.tile([128, ST, EH], f32r)   # x[:, 0:1024]
                for s in range(ST):
                    nc.sync.dma_start(
                        xh[:, s, :], x_se[s * 128:(s + 1) * 128, 0:EH])

                # -- (a) rows 0-3 x cols 0-7
                with tc.tile_pool(name="g_psa", bufs=2, space="PSUM") as psp, \
                     tc.tile_pool(name="g_sta", bufs=2) as p_sta, \
                     tc.tile_pool(name="g_tpa", bufs=2, space="PSUM") as tpa:
                    for i in range(4):
                        ps = psp.tile([128, 1024], f32, tag="ps")
                        for s in range(ST):
                            lhsT = xh[:, s, i * 128:(i + 1) * 128]
                            nc.tensor.matmul(ps[:, 0:512], lhsT,
                                             xh[:, s, 0:512],
                                             start=(s == 0), stop=False)
                            nc.tensor.matmul(ps[:, 512:1024], lhsT,
                                             xh[:, s, 512:1024],
                                             start=(s == 0), stop=(s == ST - 1))
                        st_ = evict(p_sta, ps, i, 0, 8, f"a{i}")
                        mirror(tpa, st_, i, 0, [4, 5, 6, 7], f"a{i}")

                # -- (b) rows 4-7 x cols 4-7 (no mirrors)
                with tc.tile_pool(name="g_psb", bufs=2, space="PSUM") as psp, \
                     tc.tile_pool(name="g_stb", bufs=2) as p_stb:
                    for i in range(4, 8):
                        ps = psp.tile([128, 512], f32, tag="ps")
                        for s in range(ST):
                            nc.tensor.matmul(
                                ps[:, :], xh[:, s, i * 128:(i + 1) * 128],
                                xh[:, s, 512:1024],
                                start=(s == 0), stop=(s == ST - 1))
                        evict(p_stb, ps, i, 4, 4, f"b{i}")

                # -- (c) rows 0-7 x cols 8-15: rhs = xR streamed (2 halves)
                for ch in range(2):
                    sts = []
                    stc_ctx = ExitStack()
                    p_stc = stc_ctx.enter_context(
                        tc.tile_pool(name=f"g_stc{ch}", bufs=8))
                    with tc.tile_pool(name=f"g_psc{ch}", bufs=1,
                                      space="PSUM") as psp, \
                         tc.tile_pool(name=f"g_cs{ch}", bufs=3) as p_cs:
                        pss = [psp.tile([128, 512], f32, tag=f"c{i}",
                                        name=f"psc{ch}_{i}")
                               for i in range(8)]
                        for s in range(ST):
                            xrt = p_cs.tile([128, 512], f32r, tag="xr")
                            nc.scalar.dma_start(
                                xrt[:, :],
                                x_se[s * 128:(s + 1) * 128,
                                     EH + ch * 512:EH + (ch + 1) * 512])
                            for i in range(8):
                                nc.tensor.matmul(
                                    pss[i][:, :],
                                    xh[:, s, i * 128:(i + 1) * 128],
                                    xrt[:, :],
                                    start=(s == 0), stop=(s == ST - 1))
                        for i in range(8):
                            sts.append(evict(p_stc, pss[i], i, 8 + 4 * ch, 4,
                                             f"c{ch}_{i}"))
                    with tc.tile_pool(name=f"g_tpc{ch}", bufs=2,
                                      space="PSUM") as tpc:
                        js = [8 + 4 * ch + t for t in range(4)]
                        for i in range(8):
                            mirror(tpc, sts[i], i, 8 + 4 * ch, js, f"c{ch}_{i}")
                    stc_ctx.close()

            # -- (d)/(e): xR resident
            with tc.tile_pool(name="g_xres2", bufs=1) as p_res2:
                xr = p_res2.tile([128, ST, EH], f32r)  # x[:, 1024:2048]
                for s in range(ST):
                    nc.sync.dma_start(
                        xr[:, s, :], x_se[s * 128:(s + 1) * 128, EH:E])
                with tc.tile_pool(name="g_psd", bufs=2, space="PSUM") as psp, \
                     tc.tile_pool(name="g_std", bufs=2) as p_std, \
                     tc.tile_pool(name="g_tpd", bufs=2, space="PSUM") as tpd:
                    for i in range(8, 12):
                        ps = psp.tile([128, 1024], f32, tag="ps")
                        for s in range(ST):
                            lhsT = xr[:, s, (i - 8) * 128:(i - 7) * 128]
                            nc.tensor.matmul(ps[:, 0:512], lhsT,
                                             xr[:, s, 0:512],
                                             start=(s == 0), stop=False)
                            nc.tensor.matmul(ps[:, 512:1024], lhsT,
                                             xr[:, s, 512:1024],
                                             start=(s == 0), stop=(s == ST - 1))
                        st_ = evict(p_std, ps, i, 8, 8, f"d{i}")
                        mirror(tpd, st_, i, 8, [12, 13, 14, 15], f"d{i}")
                with tc.tile_pool(name="g_pse", bufs=2, space="PSUM") as psp, \
                     tc.tile_pool(name="g_ste", bufs=2) as p_ste:
                    for i in range(12, 16):
                        ps = psp.tile([128, 512], f32, tag="ps")
                        for s in range(ST):
                            nc.tensor.matmul(
                                ps[:, :], xr[:, s, (i - 8) * 128:(i - 7) * 128],
                                xr[:, s, 512:1024],
                                start=(s == 0), stop=(s == ST - 1))
                        evict(p_ste, ps, i, 12, 4, f"e{i}")

        # ============ Phase 2a: A^T = G-tiles @ wqt ============
        with tc.tile_pool(name="a_wq", bufs=1) as p_wq, \
             tc.tile_pool(name="a_g", bufs=3) as p_g, \
             tc.tile_pool(name="a_st", bufs=3) as p_ast, \
             tc.tile_pool(name="a_ps", bufs=2, space="PSUM") as p_aps:
            wqt_sb = p_wq.tile([128, EB, EH], f32r)
            for e in range(EB):
                nc.sync.dma_start(wqt_sb[:, e, :],
                                  wqt[e * 128:(e + 1) * 128, :])
            for fb in range(EB):
                g_sb = p_g.tile([128, EB, 128], f32r, tag="g")
                nc.scalar.dma_start(
                    g_sb[:, :, :], g_d[fb].rearrange("e p f -> p e f"))
                ps = p_aps.tile([128, 1024], f32, tag="ps")
                for e in range(EB):
                    lhsT = g_sb[:, e, :]
                    nc.tensor.matmul(ps[:, 0:512], lhsT, wqt_sb[:, e, 0:512],
                                     start=(e == 0), stop=False)
                    nc.tensor.matmul(ps[:, 512:1024], lhsT,
                                     wqt_sb[:, e, 512:1024],
                                     start=(e == 0), stop=(e == EB - 1))
                st_ = p_ast.tile([128, 1024], f32r, tag="ast")
                nc.scalar.copy(st_[:, :], ps[:, :])
                nc.sync.dma_start(
                    at_d[fb].rearrange("mb p m -> p mb m"),
                    st_[:, :].rearrange("p (mb m) -> p mb m", m=128),
                )

        # ==== Phase 2b: scores = A^T-tiles @ wkt + bias; fused softmax ====
        with tc.tile_pool(name="b_wk", bufs=1) as p_wk, \
             tc.tile_pool(name="b_at", bufs=2) as p_at, \
             tc.tile_pool(name="b_sm", bufs=4) as p_sm, \
             tc.tile_pool(name="b_psA", bufs=2, space="PSUM") as p_psA, \
             tc.tile_pool(name="b_psB", bufs=2, space="PSUM") as p_psB:
            wkt_sb = p_wk.tile([128, EB, E], f32r)
            for e in range(EB):
                nc.sync.dma_start(wkt_sb[:, e, :],
                                  wkt[e * 128:(e + 1) * 128, :])
            for mb in range(MB):
                at_sb = p_at.tile([128, EB, 128], f32r, tag="at")
                nc.scalar.dma_start(
                    at_sb[:, :, :], at_d[:, mb].rearrange("e p m -> p e m"))
                psA = p_psA.tile([128, 1024], f32, tag="psA")
                psB = p_psB.tile([128, 1024], f32, tag="psB")
                for e in range(EB):
                    lhsT = at_sb[:, e, :]
                    nc.tensor.matmul(psA[:, 0:512], lhsT, wkt_sb[:, e, 0:512],
                                     start=(e == 0), stop=False)
                    nc.tensor.matmul(psA[:, 512:1024], lhsT,
                                     wkt_sb[:, e, 512:1024],
                                     start=(e == 0), stop=False)
                    nc.tensor.matmul(psB[:, 0:512], lhsT,
                                     wkt_sb[:, e, 1024:1536],
                                     start=(e == 0), stop=False)
                    nc.tensor.matmul(psB[:, 512:1024], lhsT,
                                     wkt_sb[:, e, 1536:2048],
                                     start=(e == 0), stop=False)
                ub = u_sb[:, mb * 128:(mb + 1) * 128]
                nc.tensor.matmul(psA[:, 0:512], ub, w_sb[:, 0:512],
                                 start=False, stop=True)
                nc.tensor.matmul(psA[:, 512:1024], ub, w_sb[:, 512:1024],
                                 start=False, stop=True)
                nc.tensor.matmul(psB[:, 0:512], ub, w_sb[:, 1024:1536],
                                 start=False, stop=True)
                nc.tensor.matmul(psB[:, 512:1024], ub, w_sb[:, 1536:2048],
                                 start=False, stop=True)
                # fused softmax over the f axis (2048 = two psum tiles)
                negA = p_sm.tile([128, 1], f32, tag="negA")
                negB = p_sm.tile([128, 1], f32, tag="negB")
                nc.vector.tensor_reduce(out=negA[:, :], in_=psA[:, :],
                                        op=Alu.max, axis=Ax.X, negate=True)
                nc.vector.tensor_reduce(out=negB[:, :], in_=psB[:, :],
                                        op=Alu.max, axis=Ax.X, negate=True)
                negm = p_sm.tile([128, 1], f32, tag="negm")
                nc.vector.tensor_scalar(out=negm[:, :], in0=negA[:, :],
                                        scalar1=negB[:, 0:1], scalar2=None,
                                        op0=Alu.min)
                sumA = p_sm.tile([128, 1], f32, tag="sumA")
                sumB = p_sm.tile([128, 1], f32, tag="sumB")
                attnA = p_sm.tile([128, 1024], f32r, tag="attnA")
                attnB = p_sm.tile([128, 1024], f32r, tag="attnB")
                nc.scalar.activation(attnA[:, :], psA[:, :], Act.Exp,
                                     bias=negm[:, 0:1], scale=1.0,
                                     accum_out=sumA[:, 0:1])
                nc.scalar.activation(attnB[:, :], psB[:, :], Act.Exp,
                                     bias=negm[:, 0:1], scale=1.0,
                                     accum_out=sumB[:, 0:1])
                ssum = p_sm.tile([128, 1], f32, tag="ssum")
                nc.vector.tensor_scalar(out=ssum[:, :], in0=sumA[:, :],
                                        scalar1=sumB[:, 0:1], scalar2=None,
                                        op0=Alu.add)
                nc.vector.reciprocal(rsum_sb[:, mb:mb + 1], ssum[:, :])
                nc.sync.dma_start(attn_d[mb, :, 0:1024], attnA[:, :])
                nc.sync.dma_start(attn_d[mb, :, 1024:2048], attnB[:, :])

        # ===== Phase P: attn^T (PE transpose), P^T = Wv-tiles @ attn^T =====
        with tc.tile_pool(name="p_res", bufs=1) as p_pres:
            pT_sb = p_pres.tile([128, EB, EH], f32r)     # [e-part, eb, m]
            att_ctx = ExitStack()
            p_attres = att_ctx.enter_context(
                tc.tile_pool(name="p_attres", bufs=1))
            attnT_sb = p_attres.tile([128, EB, EH], f32r)  # [f-part, fb, m]
            with tc.tile_pool(name="p_ld", bufs=2) as p_ld, \
                 tc.tile_pool(name="p_tps", bufs=4, space="PSUM") as p_tps:
                for mb in range(MB):
                    ld = p_ld.tile([128, E], f32r, tag="ld")
                    nc.scalar.dma_start(ld[:, :], attn_d[mb])
                    for g in range(4):
                        pst = p_tps.tile([128, 512], f32r, tag="pst")
                        for t in range(4):
                            fkt = 4 * g + t
                            nc.tensor.transpose(
                                pst[:, t * 128:(t + 1) * 128],
                                ld[:, fkt * 128:(fkt + 1) * 128],
                                ident[:, :])
                        nc.vector.tensor_copy(
                            attnT_sb[:, 4 * g:4 * g + 4,
                                     mb * 128:(mb + 1) * 128],
                            pst[:, :].rearrange("p (c f) -> p c f", f=128))
            # attn @ bv (unnormalized) via K=128 accumulating matmuls, N=1
            with tc.tile_pool(name="p_bv", bufs=2, space="PSUM") as p_bvp:
                for mb in range(MB):
                    psbv = p_bvp.tile([128, 256], f32, tag="psbv")
                    for fkt in range(EB):
                        nc.tensor.matmul(
                            psbv[:, :],
                            attnT_sb[:, fkt, mb * 128:(mb + 1) * 128],
                            bv_sb[:, fkt, :],
                            start=(fkt == 0), stop=(fkt == EB - 1))
                    nc.vector.tensor_copy(abv_sb[:, mb:mb + 1], psbv[:, 0:1])
            with tc.tile_pool(name="p_wv", bufs=3) as p_wv, \
                 tc.tile_pool(name="p_ps", bufs=2, space="PSUM") as p_pps:
                for eb in range(EB):
                    wv_sb = p_wv.tile([128, EB, 128], f32r, tag="wv")
                    nc.scalar.dma_start(
                        wv_sb[:, :, :], wv_t[eb].rearrange("f p e -> p f e"))
                    ps = p_pps.tile([128, 1024], f32, tag="ps")
                    for fkt in range(EB):
                        lhsT = wv_sb[:, fkt, :]
                        nc.tensor.matmul(ps[:, 0:512], lhsT,
                                         attnT_sb[:, fkt, 0:512],
                                         start=(fkt == 0), stop=False)
                        nc.tensor.matmul(ps[:, 512:1024], lhsT,
                                         attnT_sb[:, fkt, 512:1024],
                                         start=(fkt == 0),
                                         stop=(fkt == EB - 1))
                    nc.scalar.copy(pT_sb[:, eb, :], ps[:, :])
            att_ctx.close()  # free attnT_sb before the out phase

            # ========= Phase out: out = P^T-tiles @ x^T =========
            for mb in range(MB):
                nc.vector.tensor_scalar(
                    out=biasf_sb[:, mb:mb + 1], in0=abv_sb[:, mb:mb + 1],
                    scalar1=rsum_sb[:, mb:mb + 1], scalar2=None, op0=Alu.mult)
            with tc.tile_pool(name="o_xt", bufs=2) as p_xt, \
                 tc.tile_pool(name="o_st", bufs=4) as p_ost, \
                 tc.tile_pool(name="o_ps", bufs=4, space="PSUM") as p_ops:
                for sc in range(8):
                    xt_sb = p_xt.tile([128, EB, 512], f32r, tag="xt")
                    nc.scalar.dma_start(
                        xt_sb[:, :, :],
                        xt[:, sc * 512:(sc + 1) * 512].rearrange(
                            "(e p) s -> p e s", p=128))
                    for mb in range(MB):
                        ps = p_ops.tile([128, 512], f32, tag="ps")
                        for e in range(EB):
                            nc.tensor.matmul(
                                ps[:, :],
                                pT_sb[:, e, mb * 128:(mb + 1) * 128],
                                xt_sb[:, e, :],
                                start=(e == 0), stop=(e == EB - 1))
                        osb = p_ost.tile([128, 512], f32, tag="osb")
                        nc.scalar.activation(
                            osb[:, :], ps[:, :], Act.Identity,
                            bias=biasf_sb[:, mb:mb + 1],
                            scale=rsum_sb[:, mb:mb + 1])
                        nc.sync.dma_start(
                            outt[mb * 128:(mb + 1) * 128,
                                 sc * 512:(sc + 1) * 512],
                            osb[:, :])

    nc.compile()
    return nc


_NC_CACHE = {}


def _get_nc():
    if "nc" not in _NC_CACHE:
        _NC_CACHE["nc"] = build_kernel()
    return _NC_CACHE["nc"]


def make_in_maps(x, Wq, bq, Wk, bk, Wv, bv):
    sc = np.float32(1.0 / np.sqrt(E))
    x = np.asarray(x, np.float32)
    Wq = np.asarray(Wq, np.float32)
    bq = np.asarray(bq, np.float32)
    Wk = np.asarray(Wk, np.float32)
    bk = np.asarray(bk, np.float32)
    Wv = np.asarray(Wv, np.float32)
    bv = np.asarray(bv, np.float32)

    wkt_s = np.ascontiguousarray(Wk.T)                      # [E, E]
    wv_tiled = np.ascontiguousarray(
        Wv.reshape(EB, 128, EB, 128).transpose(2, 0, 1, 3)  # [eb][fb][f][e]
    )
    bv_pack = np.ascontiguousarray(
        np.broadcast_to(bv.reshape(EB, 128).T[:, :, None],
                        (128, EB, 256)))                     # [128, EB, 256]

    per_batch = []
    for b in range(B):
        xb = np.ascontiguousarray(x[b])                     # [S, E]
        xtb = np.ascontiguousarray(x[b].T)                  # [E, S]
        r = xb.sum(axis=0, dtype=np.float64).astype(np.float32)  # [E]
        kr = (Wk @ r).astype(np.float32)                    # [E]
        per_batch.append((xb, xtb, r, kr))

    in_maps = []
    for c in range(N_CORES):
        b, h = c // 2, c % 2
        xb, xtb, r, kr = per_batch[b]
        wq_h = Wq[h * EH:(h + 1) * EH, :] * sc              # [EH, E]
        wqt_h = np.ascontiguousarray(wq_h.T)                # [E, EH]
        btq = bq[h * EH:(h + 1) * EH] * sc
        qr = (wq_h @ r).astype(np.float32)
        u = np.ascontiguousarray(
            np.stack([btq, qr + np.float32(S) * btq]))      # [2, EH]
        w = np.ascontiguousarray(np.stack([kr, bk]))        # [2, E]
        in_maps.append({
            "x_se": xb,
            "xt": xtb,
            "wqt": wqt_h,
            "wkt": wkt_s,
            "wv_t": wv_tiled,
            "u_in": u,
            "w_in": w,
            "bv_in": bv_pack,
        })
    return in_maps


def run(in_maps, trace=False, **kwargs):
    nc = _get_nc()
    return run_bass_kernel_spmd(
        nc, in_maps, core_ids=list(range(N_CORES)), trace=trace, **kwargs
    )


def kernel(x, Wq, bq, Wk, bk, Wv, bv):
    in_maps = make_in_maps(x, Wq, bq, Wk, bk, Wv, bv)
    res = run(in_maps, trace=False)
    out = np.empty((B, E, S), dtype=np.float32)
    for c in range(N_CORES):
        b, h = c // 2, c % 2
        out[b, h * EH:(h + 1) * EH, :] = res.results[c]["outt"]
    return out


# revision 13
# speedup vs baseline: 1.6905x; 1.0904x over previous
"""Trainium2 Bass kernel for nn_AttentionModel (B=4, S=4096, E=2048) on 8 cores.

Sharding: data-parallel over batch B (4) x tensor-parallel over the E output
dim of the Q projection (2). Core c handles batch b=c//2 and scores rows
m in [h*1024, (h+1)*1024) with h=c%2. No collectives.

Two reassociations cut per-core FLOPs from 120G (qkv+scores+out) to 64.5G:

  scores = (Wq_h/sqrt(E)) @ G @ Wk^T + rank-2 bias term,
  with G = x^T x the symmetric Gram matrix: only 160/256 of its 128x128
  tiles are computed on the PE; the 96 mirror tiles are PE transposes.
  This replaces the q and k projections and the [m,S]x[S,f] score GEMM.

  out = attn @ v = (attn @ Wv) @ x^T + (attn.bv): the v projection is never
  materialized. P^T = Wv-tiles @ attn^T costs 8.6G vs v's 34.4G.

Softmax keeps exp() unnormalized through the P GEMM; 1/sum and the attn.bv
bias fold into the final eviction as per-partition scale/bias.

Bias correction (host-precomputed, r = sum_s x[s,:]):
  scores += btq (x) kr + (qr + S*btq) (x) bk,  btq = bq_h/sqrt(E),
  qr = (Wq_h/sqrt(E)) r, kr = Wk r  -- a single K=2 matmul accumulation.

Schedule keeps the PE stream dense (HAM throttles on >3.4us idle): mirror
transposes run as region-end passes over staged tiles, phase 2a is split in
half around the G (d)/(e) sweeps so the second x-half residency load hides
behind 2a compute, and region (c) reuses one PSUM pool across both column
halves so the boundary pipelines.
"""

import sys

sys.path.insert(0, "/opt/trn_rl_repo")

from contextlib import ExitStack

import numpy as np

import concourse.bass as bass
import concourse.mybir as mybir
import concourse.tile as tile
from concourse import bacc
from concourse.bass_utils import run_bass_kernel_spmd
from concourse.masks import make_identity

f32 = mybir.dt.float32
f32r = mybir.dt.float32r

B, S, E = 4, 4096, 2048
EH = E // 2          # per-core scores rows (embed half)
EB = E // 128        # 16 e-blocks
MB = EH // 128       # 8 m-blocks
ST = S // 128        # 32 s-chunks
N_CORES = 8

Act = mybir.ActivationFunctionType
Alu = mybir.AluOpType
Ax = mybir.AxisListType


def build_kernel():
    nc = bacc.Bacc("TRN2", debug=False, target_bir_lowering=False)

    x_se = nc.dram_tensor("x_se", [S, E], f32r, kind="ExternalInput")   # x[b]
    xt = nc.dram_tensor("xt", [E, S], f32r, kind="ExternalInput")       # x[b]^T
    wqt = nc.dram_tensor("wqt", [E, EH], f32r, kind="ExternalInput")    # (Wq_h/sqrtE)^T
    wkt = nc.dram_tensor("wkt", [E, E], f32r, kind="ExternalInput")     # Wk^T
    wv_t = nc.dram_tensor("wv_t", [EB, EB, 128, 128], f32r, kind="ExternalInput")
    u_in = nc.dram_tensor("u_in", [2, EH], f32r, kind="ExternalInput")  # [btq; qr+S*btq]
    w_in = nc.dram_tensor("w_in", [2, E], f32r, kind="ExternalInput")   # [kr; bk]
    bv_in = nc.dram_tensor("bv_in", [128, EB, 256], f32r, kind="ExternalInput")
    outt = nc.dram_tensor("outt", [EH, S], f32, kind="ExternalOutput")

    with tile.TileContext(nc) as tc, ExitStack() as ctx:
        dram = ctx.enter_context(tc.tile_pool(name="dram", bufs=1, space="DRAM"))
        g_d = dram.tile([EB, EB, 128, 128], f32r)     # [fb][e] = G[e-blk, fb-blk]
        at_d = dram.tile([EB, MB, 128, 128], f32r)    # [e'b][mb] = A^T tile
        attn_d = dram.tile([MB, 128, E], f32r)        # unnormalized exp rows

        const = ctx.enter_context(tc.tile_pool(name="const", bufs=1))
        ident_f = const.tile([128, 128], f32)
        make_identity(nc, ident_f[:, :])
        ident = const.tile([128, 128], f32r)
        nc.scalar.copy(ident[:, :], ident_f[:, :])

        keep = ctx.enter_context(tc.tile_pool(name="keep", bufs=1))
        rsum_sb = keep.tile([128, MB], f32)      # 1/softmax-sum per m-block col
        abv_sb = keep.tile([128, MB], f32)       # attn@bv (unnormalized)
        biasf_sb = keep.tile([128, MB], f32)     # abv*rsum

        # ================= Phase G: symmetric Gram =================
        # direct regions (tile (row i, col j) of G):
        #   (a) i 0-3,  j 0-7    (b) i 4-7, j 4-7     (c) i 0-7, j 8-15
        #   (d1) i 8-11, j 8-11  (d2) i 8-11, j 12-15 (e) i 12-15, j 12-15
        # mirror transposes: T(a:j4-7)->(4-7,0-3); T(c)->(8-15,0-7);
        #   T(d2)->(12-15,8-11).  Mirrors run as region-end passes.
        with tc.tile_pool(name="g_tstage", bufs=4) as p_tst:
            n_ev = [0]

            def evict(p_gst, ps, i, j0, jn, tag, bufs=None):
                # ps [128, jn*128] = G[i-block, j0:j0+jn) -> g_d[j][i]
                st_ = p_gst.tile([128, jn * 128], f32r, tag="gst",
                                 name=f"gst_{tag}", bufs=bufs)
                n_ev[0] += 1
                eng = nc.scalar if n_ev[0] % 2 else nc.vector
                if eng is nc.scalar:
                    nc.scalar.copy(st_[:, :], ps[:, :])
                else:
                    nc.vector.tensor_copy(st_[:, :], ps[:, :])
                nc.sync.dma_start(
                    g_d[j0:j0 + jn, i].rearrange("j p e -> p j e"),
                    st_[:, :].rearrange("p (j e) -> p j e", e=128),
                )
                return st_

            def mirror(tp, st_, i, j0, js, tag):
                # write T(G[i, j]) -> g_d[i][j] for j in js (js contiguous)
                pst = tp.tile([128, 128 * len(js)], f32r, tag="tps",
                              name=f"tps_{tag}")
                for t, j in enumerate(js):
                    nc.tensor.transpose(
                        pst[:, t * 128:(t + 1) * 128],
                        st_[:, (j - j0) * 128:(j - j0 + 1) * 128],
                        ident[:, :],
                    )
                ts_ = p_tst.tile([128, 128 * len(js)], f32r, tag="tst",
                                 name=f"tst_{tag}")
                nc.vector.tensor_copy(ts_[:, :], pst[:, :])
                nc.sync.dma_start(
                    g_d[i, js[0]:js[0] + len(js)].rearrange("j p e -> p j e"),
                    ts_[:, :].rearrange("p (j e) -> p j e", e=128),
                )

            with tc.tile_pool(name="g_xres", bufs=1) as p_res:
                xh = p_res.tile([128, ST, EH], f32r)   # x[:, 0:1024]
                for s in range(ST):
                    nc.sync.dma_start(
                        xh[:, s, :], x_se[s * 128:(s + 1) * 128, 0:EH])

                # -- (a) + (b) sweeps, then T-a pass
                with tc.tile_pool(name="g_psa", bufs=2, space="PSUM") as psa, \
                     tc.tile_pool(name="g_sta", bufs=4) as p_sta, \
                     tc.tile_pool(name="g_psb", bufs=2, space="PSUM") as psb, \
                     tc.tile_pool(name="g_stb", bufs=2) as p_stb, \
                     tc.tile_pool(name="g_tpa", bufs=2, space="PSUM") as tpa:
                    sta = []
                    for i in range(4):
                        ps = psa.tile([128, 1024], f32, tag="ps")
                        for s in range(ST):
                            lhsT = xh[:, s, i * 128:(i + 1) * 128]
                            nc.tensor.matmul(ps[:, 0:512], lhsT,
                                             xh[:, s, 0:512],
                                             start=(s == 0), stop=False)
                            nc.tensor.matmul(ps[:, 512:1024], lhsT,
                                             xh[:, s, 512:1024],
                                             start=(s == 0), stop=(s == ST - 1))
                        sta.append(evict(p_sta, ps, i, 0, 8, f"a{i}"))
                    for i in range(4, 8):
                        ps = psb.tile([128, 512], f32, tag="ps")
                        for s in range(ST):
                            nc.tensor.matmul(
                                ps[:, :], xh[:, s, i * 128:(i + 1) * 128],
                                xh[:, s, 512:1024],
                                start=(s == 0), stop=(s == ST - 1))
                        evict(p_stb, ps, i, 4, 4, f"b{i}")
                    for i in range(4):
                        mirror(tpa, sta[i], i, 0, [4, 5, 6, 7], f"a{i}")

                # -- (c) rows 0-7 x cols 8-15: one PSUM pool, both halves
                stc = []
                with tc.tile_pool(name="g_stc", bufs=16) as p_stc:
                    with tc.tile_pool(name="g_psc", bufs=1,
                                      space="PSUM") as psc, \
                         tc.tile_pool(name="g_cs", bufs=3) as p_cs:
                        for ch in range(2):
                            pss = [psc.tile([128, 512], f32, tag=f"c{i}",
                                            name=f"psc{ch}_{i}")
                                   for i in range(8)]
                            for s in range(ST):
                                xrt = p_cs.tile([128, 512], f32r, tag="xr")
                                nc.scalar.dma_start(
                                    xrt[:, :],
                                    x_se[s * 128:(s + 1) * 128,
                                         EH + ch * 512:EH + (ch + 1) * 512])
                                for i in range(8):
                                    nc.tensor.matmul(
                                        pss[i][:, :],
                                        xh[:, s, i * 128:(i + 1) * 128],
                                        xrt[:, :],
                                        start=(s == 0), stop=(s == ST - 1))
                            for i in range(8):
                                stc.append(
                                    (evict(p_stc, pss[i], i, 8 + 4 * ch, 4,
                                           f"c{ch}_{i}"), i, 8 + 4 * ch))
                    with tc.tile_pool(name="g_tpc", bufs=2,
                                      space="PSUM") as tpc:
                        for st_, i, j0 in stc:
                            mirror(tpc, st_, i, j0,
                                   [j0, j0 + 1, j0 + 2, j0 + 3],
                                   f"c{j0}_{i}")

            # ---- 2a half 1 (fb 0-7) while xr_a loads ----
            with tc.tile_pool(name="g_xra", bufs=1) as p_xra:
                xra = p_xra.tile([128, ST, 512], f32r)  # x[:, 1024:1536]
                for s in range(ST):
                    nc.sync.dma_start(
                        xra[:, s, :],
                        x_se[s * 128:(s + 1) * 128, EH:EH + 512])
                phase_2a(nc, tc, g_d, at_d, wqt, range(0, EB // 2), "h1")

                # ---- (d1) rows 8-11 x cols 8-11, xr_b loads behind it ----
                with tc.tile_pool(name="g_xrb", bufs=1) as p_xrb:
                    xrb = p_xrb.tile([128, ST, 512], f32r)  # x[:, 1536:2048]
                    for s in range(ST):
                        nc.sync.dma_start(
                            xrb[:, s, :],
                            x_se[s * 128:(s + 1) * 128, EH + 512:E])
                    with tc.tile_pool(name="g_psd1", bufs=2,
                                      space="PSUM") as psd1, \
                         tc.tile_pool(name="g_std1", bufs=2) as p_std1:
                        for i in range(8, 12):
                            ps = psd1.tile([128, 512], f32, tag="ps")
                            for s in range(ST):
                                nc.tensor.matmul(
                                    ps[:, :],
                                    xra[:, s, (i - 8) * 128:(i - 7) * 128],
                                    xra[:, s, :],
                                    start=(s == 0), stop=(s == ST - 1))
                            evict(p_std1, ps, i, 8, 4, f"d1_{i}")
                    # (d2) rows 8-11 x cols 12-15 + (e) 12-15 x 12-15 + T-d2
                    with tc.tile_pool(name="g_psd2", bufs=2,
                                      space="PSUM") as psd2, \
                         tc.tile_pool(name="g_std2", bufs=4) as p_std2, \
                         tc.tile_pool(name="g_pse", bufs=2,
                                      space="PSUM") as pse, \
                         tc.tile_pool(name="g_ste", bufs=2) as p_ste, \
                         tc.tile_pool(name="g_tpd", bufs=2,
                                      space="PSUM") as tpd:
                        std2 = []
                        for i in range(8, 12):
                            ps = psd2.tile([128, 512], f32, tag="ps")
                            for s in range(ST):
                                nc.tensor.matmul(
                                    ps[:, :],
                                    xra[:, s, (i - 8) * 128:(i - 7) * 128],
                                    xrb[:, s, :],
                                    start=(s == 0), stop=(s == ST - 1))
                            std2.append(evict(p_std2, ps, i, 12, 4, f"d2_{i}"))
                        for i in range(12, 16):
                            ps = pse.tile([128, 512], f32, tag="ps")
                            for s in range(ST):
                                nc.tensor.matmul(
                                    ps[:, :],
                                    xrb[:, s, (i - 12) * 128:(i - 11) * 128],
                                    xrb[:, s, :],
                                    start=(s == 0), stop=(s == ST - 1))
                            evict(p_ste, ps, i, 12, 4, f"e{i}")
                        for t, i in enumerate(range(8, 12)):
                            mirror(tpd, std2[t], i, 12, [12, 13, 14, 15],
                                   f"d2_{i}")

        # ---- 2a half 2 (fb 8-15) ----
        phase_2a(nc, tc, g_d, at_d, wqt, range(EB // 2, EB), "h2")

        # ==== Phase 2b: scores = A^T-tiles @ wkt + bias; fused softmax ====
        with tc.tile_pool(name="b_wk", bufs=1) as p_wk, \
             tc.tile_pool(name="b_uw", bufs=1) as p_uw, \
             tc.tile_pool(name="b_at", bufs=2) as p_at, \
             tc.tile_pool(name="b_sm", bufs=4) as p_sm, \
             tc.tile_pool(name="b_psA", bufs=2, space="PSUM") as p_psA, \
             tc.tile_pool(name="b_psB", bufs=2, space="PSUM") as p_psB:
            wkt_sb = p_wk.tile([128, EB, E], f32r)
            for e in range(EB):
                nc.sync.dma_start(wkt_sb[:, e, :],
                                  wkt[e * 128:(e + 1) * 128, :])
            u_sb = p_uw.tile([2, EH], f32r)
            nc.sync.dma_start(u_sb[:, :], u_in[:, :])
            w_sb = p_uw.tile([2, E], f32r)
            nc.sync.dma_start(w_sb[:, :], w_in[:, :])
            for mb in range(MB):
                at_sb = p_at.tile([128, EB, 128], f32r, tag="at")
                nc.scalar.dma_start(
                    at_sb[:, :, :], at_d[:, mb].rearrange("e p m -> p e m"))
                psA = p_psA.tile([128, 1024], f32, tag="psA")
                psB = p_psB.tile([128, 1024], f32, tag="psB")
                for e in range(EB):
                    lhsT = at_sb[:, e, :]
                    nc.tensor.matmul(psA[:, 0:512], lhsT, wkt_sb[:, e, 0:512],
                                     start=(e == 0), stop=False)
                    nc.tensor.matmul(psA[:, 512:1024], lhsT,
                                     wkt_sb[:, e, 512:1024],
                                     start=(e == 0), stop=False)
                    nc.tensor.matmul(psB[:, 0:512], lhsT,
                                     wkt_sb[:, e, 1024:1536],
                                     start=(e == 0), stop=False)
                    nc.tensor.matmul(psB[:, 512:1024], lhsT,
                                     wkt_sb[:, e, 1536:2048],
                                     start=(e == 0), stop=False)
                ub = u_sb[:, mb * 128:(mb + 1) * 128]
                nc.tensor.matmul(psA[:, 0:512], ub, w_sb[:, 0:512],
                                 start=False, stop=True)
                nc.tensor.matmul(psA[:, 512:1024], ub, w_sb[:, 512:1024],
                                 start=False, stop=True)
                nc.tensor.matmul(psB[:, 0:512], ub, w_sb[:, 1024:1536],
                                 start=False, stop=True)
                nc.tensor.matmul(psB[:, 512:1024], ub, w_sb[:, 1536:2048],
                                 start=False, stop=True)
                # fused softmax over the f axis (2048 = two psum tiles)
                negA = p_sm.tile([128, 1], f32, tag="negA")
                negB = p_sm.tile([128, 1], f32, tag="negB")
                nc.vector.tensor_reduce(out=negA[:, :], in_=psA[:, :],
                                        op=Alu.max, axis=Ax.X, negate=True)
                nc.vector.tensor_reduce(out=negB[:, :], in_=psB[:, :],
                                        op=Alu.max, axis=Ax.X, negate=True)
                negm = p_sm.tile([128, 1], f32, tag="negm")
                nc.vector.tensor_scalar(out=negm[:, :], in0=negA[:, :],
                                        scalar1=negB[:, 0:1], scalar2=None,
                                        op0=Alu.min)
                sumA = p_sm.tile([128, 1], f32, tag="sumA")
                sumB = p_sm.tile([128, 1], f32, tag="sumB")
                attnA = p_sm.tile([128, 1024], f32r, tag="attnA")
                attnB = p_sm.tile([128, 1024], f32r, tag="attnB")
                nc.scalar.activation(attnA[:, :], psA[:, :], Act.Exp,
                                     bias=negm[:, 0:1], scale=1.0,
                                     accum_out=sumA[:, 0:1])
                nc.scalar.activation(attnB[:, :], psB[:, :], Act.Exp,
                                     bias=negm[:, 0:1], scale=1.0,
                                     accum_out=sumB[:, 0:1])
                ssum = p_sm.tile([128, 1], f32, tag="ssum")
                nc.vector.tensor_scalar(out=ssum[:, :], in0=sumA[:, :],
                                        scalar1=sumB[:, 0:1], scalar2=None,
                                        op0=Alu.add)
                nc.vector.reciprocal(rsum_sb[:, mb:mb + 1], ssum[:, :])
                nc.sync.dma_start(attn_d[mb, :, 0:1024], attnA[:, :])
                nc.sync.dma_start(attn_d[mb, :, 1024:2048], attnB[:, :])

        # ===== Phase P: attn^T (PE transpose), P^T = Wv-tiles @ attn^T =====
        with tc.tile_pool(name="p_res", bufs=1) as p_pres:
            pT_sb = p_pres.tile([128, EB, EH], f32r)     # [e-part, eb, m]
            att_ctx = ExitStack()
            p_attres = att_ctx.enter_context(
                tc.tile_pool(name="p_attres", bufs=1))
            attnT_sb = p_attres.tile([128, EB, EH], f32r)  # [f-part, fb, m]
            with tc.tile_pool(name="p_ld", bufs=2) as p_ld, \
                 tc.tile_pool(name="p_tps", bufs=4, space="PSUM") as p_tps:
                for mb in range(MB):
                    ld = p_ld.tile([128, E], f32r, tag="ld")
                    nc.scalar.dma_start(ld[:, :], attn_d[mb])
                    for g in range(4):
                        pst = p_tps.tile([128, 512], f32r, tag="pst")
                        for t in range(4):
                            fkt = 4 * g + t
                            nc.tensor.transpose(
                                pst[:, t * 128:(t + 1) * 128],
                                ld[:, fkt * 128:(fkt + 1) * 128],
                                ident[:, :])
                        nc.vector.tensor_copy(
                            attnT_sb[:, 4 * g:4 * g + 4,
                                     mb * 128:(mb + 1) * 128],
                            pst[:, :].rearrange("p (c f) -> p c f", f=128))
            # attn @ bv (unnormalized)
            with tc.tile_pool(name="p_bvc", bufs=1) as p_bvc, \
                 tc.tile_pool(name="p_bv", bufs=2, space="PSUM") as p_bvp:
                bv_sb = p_bvc.tile([128, EB, 256], f32r)
                nc.sync.dma_start(bv_sb[:, :, :], bv_in[:, :, :])
                for mb in range(MB):
                    psbv = p_bvp.tile([128, 256], f32, tag="psbv")
                    for fkt in range(EB):
                        nc.tensor.matmul(
                            psbv[:, :],
                            attnT_sb[:, fkt, mb * 128:(mb + 1) * 128],
                            bv_sb[:, fkt, :],
                            start=(fkt == 0), stop=(fkt == EB - 1))
                    nc.vector.tensor_copy(abv_sb[:, mb:mb + 1], psbv[:, 0:1])
            with tc.tile_pool(name="p_wv", bufs=3) as p_wv, \
                 tc.tile_pool(name="p_ps", bufs=2, space="PSUM") as p_pps:
                for eb in range(EB):
                    wv_sb = p_wv.tile([128, EB, 128], f32r, tag="wv")
                    nc.scalar.dma_start(
                        wv_sb[:, :, :], wv_t[eb].rearrange("f p e -> p f e"))
                    ps = p_pps.tile([128, 1024], f32, tag="ps")
                    for fkt in range(EB):
                        lhsT = wv_sb[:, fkt, :]
                        nc.tensor.matmul(ps[:, 0:512], lhsT,
                                         attnT_sb[:, fkt, 0:512],
                                         start=(fkt == 0), stop=False)
                        nc.tensor.matmul(ps[:, 512:1024], lhsT,
                                         attnT_sb[:, fkt, 512:1024],
                                         start=(fkt == 0),
                                         stop=(fkt == EB - 1))
                    nc.scalar.copy(pT_sb[:, eb, :], ps[:, :])
            att_ctx.close()  # free attnT_sb before the out phase

            # ========= Phase out: out = P^T-tiles @ x^T =========
            for mb in range(MB):
                nc.vector.tensor_scalar(
                    out=biasf_sb[:, mb:mb + 1], in0=abv_sb[:, mb:mb + 1],
                    scalar1=rsum_sb[:, mb:mb + 1], scalar2=None, op0=Alu.mult)
            with tc.tile_pool(name="o_xt", bufs=2) as p_xt, \
                 tc.tile_pool(name="o_st", bufs=4) as p_ost, \
                 tc.tile_pool(name="o_ps", bufs=4, space="PSUM") as p_ops:
                for sc in range(8):
                    xt_sb = p_xt.tile([128, EB, 512], f32r, tag="xt")
                    nc.scalar.dma_start(
                        xt_sb[:, :, :],
                        xt[:, sc * 512:(sc + 1) * 512].rearrange(
                            "(e p) s -> p e s", p=128))
                    for mb in range(MB):
                        ps = p_ops.tile([128, 512], f32, tag="ps")
                        for e in range(EB):
                            nc.tensor.matmul(
                                ps[:, :],
                                pT_sb[:, e, mb * 128:(mb + 1) * 128],
                                xt_sb[:, e, :],
                                start=(e == 0), stop=(e == EB - 1))
                        osb = p_ost.tile([128, 512], f32, tag="osb")
                        nc.scalar.activation(
                            osb[:, :], ps[:, :], Act.Identity,
                            bias=biasf_sb[:, mb:mb + 1],
                            scale=rsum_sb[:, mb:mb + 1])
                        nc.sync.dma_start(
                            outt[mb * 128:(mb + 1) * 128,
                                 sc * 512:(sc + 1) * 512],
                            osb[:, :])

    nc.compile()
    return nc


def phase_2a(nc, tc, g_d, at_d, wqt, fb_range, suffix):
    """A^T[fb-rows, m] = sum_e G[e, fb]^T-tiles @ wqt[e, m] -> at_d[fb]."""
    with tc.tile_pool(name=f"a_wq{suffix}", bufs=1) as p_wq, \
         tc.tile_pool(name=f"a_g{suffix}", bufs=2) as p_g, \
         tc.tile_pool(name=f"a_st{suffix}", bufs=2) as p_ast, \
         tc.tile_pool(name=f"a_ps{suffix}", bufs=2, space="PSUM") as p_aps:
        wqt_sb = p_wq.tile([128, EB, EH], f32r, name=f"wqt_sb{suffix}")
        for e in range(EB):
            nc.sync.dma_start(wqt_sb[:, e, :], wqt[e * 128:(e + 1) * 128, :])
        for fb in fb_range:
            g_sb = p_g.tile([128, EB, 128], f32r, tag="g", name=f"g{suffix}")
            nc.scalar.dma_start(
                g_sb[:, :, :], g_d[fb].rearrange("e p f -> p e f"))
            ps = p_aps.tile([128, 1024], f32, tag="ps", name=f"ps{suffix}")
            for e in range(EB):
                lhsT = g_sb[:, e, :]
                nc.tensor.matmul(ps[:, 0:512], lhsT, wqt_sb[:, e, 0:512],
                                 start=(e == 0), stop=False)
                nc.tensor.matmul(ps[:, 512:1024], lhsT,
                                 wqt_sb[:, e, 512:1024],
                                 start=(e == 0), stop=(e == EB - 1))
            st_ = p_ast.tile([128, 1024], f32r, tag="ast",
                             name=f"ast{suffix}")
            nc.scalar.copy(st_[:, :], ps[:, :])
            nc.sync.dma_start(
                at_d[fb].rearrange("mb p m -> p mb m"),
                st_[:, :].rearrange("p (mb m) -> p mb m", m=128),
            )


_NC_CACHE = {}


def _get_nc():
    if "nc" not in _NC_CACHE:
        _NC_CACHE["nc"] = build_kernel()
    return _NC_CACHE["nc"]


def make_in_maps(x, Wq, bq, Wk, bk, Wv, bv):
    sc = np.float32(1.0 / np.sqrt(E))
    x = np.asarray(x, np.float32)
    Wq = np.asarray(Wq, np.float32)
    bq = np.asarray(bq, np.float32)
    Wk = np.asarray(Wk, np.float32)
    bk = np.asarray(bk, np.float32)
    Wv = np.asarray(Wv, np.float32)
    bv = np.asarray(bv, np.float32)

    wkt_s = np.ascontiguousarray(Wk.T)                      # [E, E]
    wv_tiled = np.ascontiguousarray(
        Wv.reshape(EB, 128, EB, 128).transpose(2, 0, 1, 3)  # [eb][fb][f][e]
    )
    bv_pack = np.ascontiguousarray(
        np.broadcast_to(bv.reshape(EB, 128).T[:, :, None],
                        (128, EB, 256)))                    # [128, EB, 256]

    per_batch = []
    for b in range(B):
        xb = np.ascontiguousarray(x[b])                     # [S, E]
        xtb = np.ascontiguousarray(x[b].T)                  # [E, S]
        r = xb.sum(axis=0, dtype=np.float64).astype(np.float32)  # [E]
        kr = (Wk @ r).astype(np.float32)                    # [E]
        per_batch.append((xb, xtb, r, kr))

    in_maps = []
    for c in range(N_CORES):
        b, h = c // 2, c % 2
        xb, xtb, r, kr = per_batch[b]
        wq_h = Wq[h * EH:(h + 1) * EH, :] * sc              # [EH, E]
        wqt_h = np.ascontiguousarray(wq_h.T)                # [E, EH]
        btq = bq[h * EH:(h + 1) * EH] * sc
        qr = (wq_h @ r).astype(np.float32)
        u = np.ascontiguousarray(
            np.stack([btq, qr + np.float32(S) * btq]))      # [2, EH]
        w = np.ascontiguousarray(np.stack([kr, bk]))        # [2, E]
        in_maps.append({
            "x_se": xb,
            "xt": xtb,
            "wqt": wqt_h,
            "wkt": wkt_s,
            "wv_t": wv_tiled,
            "u_in": u,
            "w_in": w,
            "bv_in": bv_pack,
        })
    return in_maps


def run(in_maps, trace=False, **kwargs):
    nc = _get_nc()
    return run_bass_kernel_spmd(
        nc, in_maps, core_ids=list(range(N_CORES)), trace=trace, **kwargs
    )


def kernel(x, Wq, bq, Wk, bk, Wv, bv):
    in_maps = make_in_maps(x, Wq, bq, Wk, bk, Wv, bv)
    res = run(in_maps, trace=False)
    out = np.empty((B, E, S), dtype=np.float32)
    for c in range(N_CORES):
        b, h = c // 2, c % 2
        out[b, h * EH:(h + 1) * EH, :] = res.results[c]["outt"]
    return out


# revision 15
# speedup vs baseline: 1.7235x; 1.0195x over previous
"""Trainium2 Bass kernel for nn_AttentionModel (B=4, S=4096, E=2048) on 8 cores.

Sharding: data-parallel over batch B (4) x tensor-parallel over the E output
dim of the Q projection (2). Core c handles batch b=c//2 and scores rows
m in [h*1024, (h+1)*1024) with h=c%2. No collectives.

Two reassociations cut per-core FLOPs from 120G (qkv+scores+out) to 64.5G:

  scores = (Wq_h/sqrt(E)) @ G @ Wk^T + rank-2 bias term,
  with G = x^T x the symmetric Gram matrix: only 160/256 of its 128x128
  tiles are computed on the PE; the 96 mirror tiles are PE transposes.
  This replaces the q and k projections and the [m,S]x[S,f] score GEMM.

  out = attn @ v = (attn @ Wv) @ x^T + (attn.bv): the v projection is never
  materialized. P^T = Wv-tiles @ attn^T costs 8.6G vs v's 34.4G.

Softmax keeps exp() unnormalized through the P GEMM; 1/sum and the attn.bv
bias fold into the final eviction as per-partition scale/bias.

Bias correction (host-precomputed, r = sum_s x[s,:]):
  scores += btq (x) kr + (qr + S*btq) (x) bk,  btq = bq_h/sqrt(E),
  qr = (Wq_h/sqrt(E)) r, kr = Wk r  -- a single K=2 matmul accumulation.

Schedule keeps the PE stream dense (HAM throttles on >3.4us idle): mirror
transposes run as region-end passes over staged tiles, phase 2a is split in
half around the G (d)/(e) sweeps so the second x-half residency load hides
behind 2a compute, and region (c) reuses one PSUM pool across both column
halves so the boundary pipelines.
"""

import sys

sys.path.insert(0, "/opt/trn_rl_repo")

from contextlib import ExitStack

import numpy as np

import concourse.bass as bass
import concourse.mybir as mybir
import concourse.tile as tile
from concourse import bacc
from concourse.bass_utils import run_bass_kernel_spmd
from concourse.masks import make_identity

f32 = mybir.dt.float32
f32r = mybir.dt.float32r

B, S, E = 4, 4096, 2048
EH = E // 2          # per-core scores rows (embed half)
EB = E // 128        # 16 e-blocks
MB = EH // 128       # 8 m-blocks
ST = S // 128        # 32 s-chunks
N_CORES = 8

Act = mybir.ActivationFunctionType
Alu = mybir.AluOpType
Ax = mybir.AxisListType


def build_kernel():
    nc = bacc.Bacc("TRN2", debug=False, target_bir_lowering=False)

    x_se = nc.dram_tensor("x_se", [S, E], f32r, kind="ExternalInput")   # x[b]
    xt = nc.dram_tensor("xt", [E, S], f32r, kind="ExternalInput")       # x[b]^T
    wqt = nc.dram_tensor("wqt", [E, EH], f32r, kind="ExternalInput")    # (Wq_h/sqrtE)^T
    wkt = nc.dram_tensor("wkt", [E, E], f32r, kind="ExternalInput")     # Wk^T
    wv_t = nc.dram_tensor("wv_t", [EB, 128, EB, 128], f32r, kind="ExternalInput")
    u_in = nc.dram_tensor("u_in", [2, EH], f32r, kind="ExternalInput")  # [btq; qr+S*btq]
    w_in = nc.dram_tensor("w_in", [2, E], f32r, kind="ExternalInput")   # [kr; bk]
    bv_in = nc.dram_tensor("bv_in", [128, EB, 256], f32r, kind="ExternalInput")
    outt = nc.dram_tensor("outt", [EH, S], f32, kind="ExternalOutput")

    with tile.TileContext(nc) as tc, ExitStack() as ctx:
        dram = ctx.enter_context(tc.tile_pool(name="dram", bufs=1, space="DRAM"))
        g_d = dram.tile([EB, 128, E], f32r)   # row-band: g_d[i] = G[i-blk, :]
        at_d = dram.tile([EB, 128, EH], f32r)  # row-band: at_d[fb] = A^T[fb-blk]
        attn_d = dram.tile([MB, 128, E], f32r)        # unnormalized exp rows

        const = ctx.enter_context(tc.tile_pool(name="const", bufs=1))
        ident_f = const.tile([128, 128], f32)
        make_identity(nc, ident_f[:, :])
        ident = const.tile([128, 128], f32r)
        nc.scalar.copy(ident[:, :], ident_f[:, :])

        keep = ctx.enter_context(tc.tile_pool(name="keep", bufs=1))
        rsum_sb = keep.tile([128, MB], f32)      # 1/softmax-sum per m-block col
        abv_sb = keep.tile([128, MB], f32)       # attn@bv (unnormalized)
        biasf_sb = keep.tile([128, MB], f32)     # abv*rsum

        # ================= Phase G: symmetric Gram =================
        # direct regions (tile (row i, col j) of G):
        #   (a) i 0-3,  j 0-7    (b) i 4-7, j 4-7     (c) i 0-7, j 8-15
        #   (d1) i 8-11, j 8-11  (d2) i 8-11, j 12-15 (e) i 12-15, j 12-15
        # mirror transposes: T(a:j4-7)->(4-7,0-3); T(c)->(8-15,0-7);
        #   T(d2)->(12-15,8-11).  Mirrors run as region-end passes.
        with tc.tile_pool(name="g_tstage", bufs=4) as p_tst:
            n_ev = [0]

            def evict(p_gst, ps, i, j0, jn, tag, bufs=None):
                # ps [128, jn*128] = G[i-block, j0:j0+jn) -> g_d[j][i]
                st_ = p_gst.tile([128, jn * 128], f32r, tag="gst",
                                 name=f"gst_{tag}", bufs=bufs)
                n_ev[0] += 1
                eng = nc.scalar if n_ev[0] % 2 else nc.vector
                if eng is nc.scalar:
                    nc.scalar.copy(st_[:, :], ps[:, :])
                else:
                    nc.vector.tensor_copy(st_[:, :], ps[:, :])
                nc.sync.dma_start(
                    g_d[i, :, j0 * 128:(j0 + jn) * 128], st_[:, :])
                return st_

            def mirror(tp, st_, i, j0, js, tag):
                # write T(G[i, j]) -> g_d[i][j] for j in js (js contiguous)
                pst = tp.tile([128, 128 * len(js)], f32r, tag="tps",
                              name=f"tps_{tag}")
                for t, j in enumerate(js):
                    nc.tensor.transpose(
                        pst[:, t * 128:(t + 1) * 128],
                        st_[:, (j - j0) * 128:(j - j0 + 1) * 128],
                        ident[:, :],
                    )
                ts_ = p_tst.tile([128, 128 * len(js)], f32r, tag="tst",
                                 name=f"tst_{tag}")
                nc.vector.tensor_copy(ts_[:, :], pst[:, :])
                nc.sync.dma_start(
                    g_d[js[0]:js[0] + len(js), :,
                        i * 128:(i + 1) * 128].rearrange("j p e -> p j e"),
                    ts_[:, :].rearrange("p (j e) -> p j e", e=128),
                )

            with tc.tile_pool(name="g_xres", bufs=1) as p_res:
                xh = p_res.tile([128, ST, EH], f32r)   # x[:, 0:1024]
                for sb in range(8):
                    nc.sync.dma_start(
                        xh[:, sb * 4:(sb + 1) * 4, :],
                        x_se[sb * 512:(sb + 1) * 512, 0:EH].rearrange(
                            "(s p) e -> p s e", p=128))

                # -- (a) + (b) sweeps, then T-a pass
                with tc.tile_pool(name="g_psa", bufs=2, space="PSUM") as psa, \
                     tc.tile_pool(name="g_sta", bufs=4) as p_sta, \
                     tc.tile_pool(name="g_psb", bufs=2, space="PSUM") as psb, \
                     tc.tile_pool(name="g_stb", bufs=2) as p_stb, \
                     tc.tile_pool(name="g_tpa", bufs=2, space="PSUM") as tpa:
                    sta = []
                    for i in range(4):
                        ps = psa.tile([128, 1024], f32, tag="ps")
                        for s in range(ST):
                            lhsT = xh[:, s, i * 128:(i + 1) * 128]
                            nc.tensor.matmul(ps[:, 0:512], lhsT,
                                             xh[:, s, 0:512],
                                             start=(s == 0), stop=False)
                            nc.tensor.matmul(ps[:, 512:1024], lhsT,
                                             xh[:, s, 512:1024],
                                             start=(s == 0), stop=(s == ST - 1))
                        sta.append(evict(p_sta, ps, i, 0, 8, f"a{i}"))
                    for i in range(4, 8):
                        ps = psb.tile([128, 512], f32, tag="ps")
                        for s in range(ST):
                            nc.tensor.matmul(
                                ps[:, :], xh[:, s, i * 128:(i + 1) * 128],
                                xh[:, s, 512:1024],
                                start=(s == 0), stop=(s == ST - 1))
                        evict(p_stb, ps, i, 4, 4, f"b{i}")
                    for i in range(4):
                        mirror(tpa, sta[i], i, 0, [4, 5, 6, 7], f"a{i}")

                # -- (c) rows 0-7 x cols 8-15: one PSUM pool, both halves
                stc = []
                with tc.tile_pool(name="g_stc", bufs=16) as p_stc:
                    with tc.tile_pool(name="g_psc", bufs=1,
                                      space="PSUM") as psc, \
                         tc.tile_pool(name="g_cs", bufs=3) as p_cs:
                        for ch in range(2):
                            pss = [psc.tile([128, 512], f32, tag=f"c{i}",
                                            name=f"psc{ch}_{i}")
                                   for i in range(8)]
                            for s in range(ST):
                                xrt = p_cs.tile([128, 512], f32r, tag="xr")
                                nc.scalar.dma_start(
                                    xrt[:, :],
                                    x_se[s * 128:(s + 1) * 128,
                                         EH + ch * 512:EH + (ch + 1) * 512])
                                for i in range(8):
                                    nc.tensor.matmul(
                                        pss[i][:, :],
                                        xh[:, s, i * 128:(i + 1) * 128],
                                        xrt[:, :],
                                        start=(s == 0), stop=(s == ST - 1))
                            for i in range(8):
                                stc.append(
                                    (evict(p_stc, pss[i], i, 8 + 4 * ch, 4,
                                           f"c{ch}_{i}"), i, 8 + 4 * ch))
                    with tc.tile_pool(name="g_tpc", bufs=2,
                                      space="PSUM") as tpc:
                        for st_, i, j0 in stc:
                            mirror(tpc, st_, i, j0,
                                   [j0, j0 + 1, j0 + 2, j0 + 3],
                                   f"c{j0}_{i}")

            # ---- 2a half 1 (fb 0-7) while xr_a loads ----
            with tc.tile_pool(name="g_xra", bufs=1) as p_xra:
                xra = p_xra.tile([128, ST, 512], f32r)  # x[:, 1024:1536]
                for sb in range(8):
                    nc.sync.dma_start(
                        xra[:, sb * 4:(sb + 1) * 4, :],
                        x_se[sb * 512:(sb + 1) * 512,
                             EH:EH + 512].rearrange("(s p) e -> p s e", p=128))
                phase_2a(nc, tc, g_d, at_d, wqt, range(0, EB // 2), "h1")

                # ---- (d1) rows 8-11 x cols 8-11, xr_b loads behind it ----
                with tc.tile_pool(name="g_xrb", bufs=1) as p_xrb:
                    xrb = p_xrb.tile([128, ST, 512], f32r)  # x[:, 1536:2048]
                    for sb in range(8):
                        nc.sync.dma_start(
                            xrb[:, sb * 4:(sb + 1) * 4, :],
                            x_se[sb * 512:(sb + 1) * 512,
                                 EH + 512:E].rearrange(
                                     "(s p) e -> p s e", p=128))
                    with tc.tile_pool(name="g_psd1", bufs=2,
                                      space="PSUM") as psd1, \
                         tc.tile_pool(name="g_std1", bufs=2) as p_std1:
                        for i in range(8, 12):
                            ps = psd1.tile([128, 512], f32, tag="ps")
                            for s in range(ST):
                                nc.tensor.matmul(
                                    ps[:, :],
                                    xra[:, s, (i - 8) * 128:(i - 7) * 128],
                                    xra[:, s, :],
                                    start=(s == 0), stop=(s == ST - 1))
                            evict(p_std1, ps, i, 8, 4, f"d1_{i}")
                    # (d2) rows 8-11 x cols 12-15 + (e) 12-15 x 12-15 + T-d2
                    with tc.tile_pool(name="g_psd2", bufs=2,
                                      space="PSUM") as psd2, \
                         tc.tile_pool(name="g_std2", bufs=4) as p_std2, \
                         tc.tile_pool(name="g_pse", bufs=2,
                                      space="PSUM") as pse, \
                         tc.tile_pool(name="g_ste", bufs=2) as p_ste, \
                         tc.tile_pool(name="g_tpd", bufs=2,
                                      space="PSUM") as tpd:
                        std2 = []
                        for i in range(8, 12):
                            ps = psd2.tile([128, 512], f32, tag="ps")
                            for s in range(ST):
                                nc.tensor.matmul(
                                    ps[:, :],
                                    xra[:, s, (i - 8) * 128:(i - 7) * 128],
                                    xrb[:, s, :],
                                    start=(s == 0), stop=(s == ST - 1))
                            std2.append(evict(p_std2, ps, i, 12, 4, f"d2_{i}"))
                        for i in range(12, 16):
                            ps = pse.tile([128, 512], f32, tag="ps")
                            for s in range(ST):
                                nc.tensor.matmul(
                                    ps[:, :],
                                    xrb[:, s, (i - 12) * 128:(i - 11) * 128],
                                    xrb[:, s, :],
                                    start=(s == 0), stop=(s == ST - 1))
                            evict(p_ste, ps, i, 12, 4, f"e{i}")
                        for t, i in enumerate(range(8, 12)):
                            mirror(tpd, std2[t], i, 12, [12, 13, 14, 15],
                                   f"d2_{i}")

        # ---- 2a half 2 (fb 8-15) ----
        phase_2a(nc, tc, g_d, at_d, wqt, range(EB // 2, EB), "h2")

        # ==== Phase 2b: scores = A^T-tiles @ wkt + bias; fused softmax ====
        with tc.tile_pool(name="b_wk", bufs=1) as p_wk, \
             tc.tile_pool(name="b_uw", bufs=1) as p_uw, \
             tc.tile_pool(name="b_at", bufs=2) as p_at, \
             tc.tile_pool(name="b_sm", bufs=4) as p_sm, \
             tc.tile_pool(name="b_psA", bufs=2, space="PSUM") as p_psA, \
             tc.tile_pool(name="b_psB", bufs=2, space="PSUM") as p_psB:
            wkt_sb = p_wk.tile([128, EB, E], f32r)
            for q in range(4):
                nc.sync.dma_start(
                    wkt_sb[:, q * 4:(q + 1) * 4, :],
                    wkt[q * 512:(q + 1) * 512, :].rearrange(
                        "(e p) f -> p e f", p=128))
            u_sb = p_uw.tile([2, EH], f32r)
            nc.sync.dma_start(u_sb[:, :], u_in[:, :])
            w_sb = p_uw.tile([2, E], f32r)
            nc.sync.dma_start(w_sb[:, :], w_in[:, :])
            for mb in range(MB):
                at_sb = p_at.tile([128, EB, 128], f32r, tag="at")
                nc.scalar.dma_start(
                    at_sb[:, :, :],
                    at_d[:, :, mb * 128:(mb + 1) * 128].rearrange(
                        "e p m -> p e m"))
                psA = p_psA.tile([128, 1024], f32, tag="psA")
                psB = p_psB.tile([128, 1024], f32, tag="psB")
                for e in range(EB):
                    lhsT = at_sb[:, e, :]
                    nc.tensor.matmul(psA[:, 0:512], lhsT, wkt_sb[:, e, 0:512],
                                     start=(e == 0), stop=False)
                    nc.tensor.matmul(psA[:, 512:1024], lhsT,
                                     wkt_sb[:, e, 512:1024],
                                     start=(e == 0), stop=False)
                    nc.tensor.matmul(psB[:, 0:512], lhsT,
                                     wkt_sb[:, e, 1024:1536],
                                     start=(e == 0), stop=False)
                    nc.tensor.matmul(psB[:, 512:1024], lhsT,
                                     wkt_sb[:, e, 1536:2048],
                                     start=(e == 0), stop=False)
                ub = u_sb[:, mb * 128:(mb + 1) * 128]
                nc.tensor.matmul(psA[:, 0:512], ub, w_sb[:, 0:512],
                                 start=False, stop=True)
                nc.tensor.matmul(psA[:, 512:1024], ub, w_sb[:, 512:1024],
                                 start=False, stop=True)
                nc.tensor.matmul(psB[:, 0:512], ub, w_sb[:, 1024:1536],
                                 start=False, stop=True)
                nc.tensor.matmul(psB[:, 512:1024], ub, w_sb[:, 1536:2048],
                                 start=False, stop=True)
                # fused softmax over the f axis (2048 = two psum tiles)
                negA = p_sm.tile([128, 1], f32, tag="negA")
                negB = p_sm.tile([128, 1], f32, tag="negB")
                nc.vector.tensor_reduce(out=negA[:, :], in_=psA[:, :],
                                        op=Alu.max, axis=Ax.X, negate=True)
                nc.vector.tensor_reduce(out=negB[:, :], in_=psB[:, :],
                                        op=Alu.max, axis=Ax.X, negate=True)
                negm = p_sm.tile([128, 1], f32, tag="negm")
                nc.vector.tensor_scalar(out=negm[:, :], in0=negA[:, :],
                                        scalar1=negB[:, 0:1], scalar2=None,
                                        op0=Alu.min)
                sumA = p_sm.tile([128, 1], f32, tag="sumA")
                sumB = p_sm.tile([128, 1], f32, tag="sumB")
                attnA = p_sm.tile([128, 1024], f32r, tag="attnA")
                attnB = p_sm.tile([128, 1024], f32r, tag="attnB")
                nc.scalar.activation(attnA[:, :], psA[:, :], Act.Exp,
                                     bias=negm[:, 0:1], scale=1.0,
                                     accum_out=sumA[:, 0:1])
                nc.scalar.activation(attnB[:, :], psB[:, :], Act.Exp,
                                     bias=negm[:, 0:1], scale=1.0,
                                     accum_out=sumB[:, 0:1])
                ssum = p_sm.tile([128, 1], f32, tag="ssum")
                nc.vector.tensor_scalar(out=ssum[:, :], in0=sumA[:, :],
                                        scalar1=sumB[:, 0:1], scalar2=None,
                                        op0=Alu.add)
                nc.vector.reciprocal(rsum_sb[:, mb:mb + 1], ssum[:, :])
                nc.sync.dma_start(attn_d[mb, :, 0:1024], attnA[:, :])
                nc.sync.dma_start(attn_d[mb, :, 1024:2048], attnB[:, :])

        # ===== Phase P: attn^T (PE transpose), P^T = Wv-tiles @ attn^T =====
        with tc.tile_pool(name="p_res", bufs=1) as p_pres:
            pT_sb = p_pres.tile([128, EB, EH], f32r)     # [e-part, eb, m]
            att_ctx = ExitStack()
            p_attres = att_ctx.enter_context(
                tc.tile_pool(name="p_attres", bufs=1))
            attnT_sb = p_attres.tile([128, EB, EH], f32r)  # [f-part, fb, m]
            with tc.tile_pool(name="p_ld", bufs=2) as p_ld, \
                 tc.tile_pool(name="p_tps", bufs=4, space="PSUM") as p_tps:
                for mb in range(MB):
                    ld = p_ld.tile([128, E], f32r, tag="ld")
                    nc.scalar.dma_start(ld[:, :], attn_d[mb])
                    for g in range(4):
                        pst = p_tps.tile([128, 512], f32r, tag="pst")
                        for t in range(4):
                            fkt = 4 * g + t
                            nc.tensor.transpose(
                                pst[:, t * 128:(t + 1) * 128],
                                ld[:, fkt * 128:(fkt + 1) * 128],
                                ident[:, :])
                        nc.vector.tensor_copy(
                            attnT_sb[:, 4 * g:4 * g + 4,
                                     mb * 128:(mb + 1) * 128],
                            pst[:, :].rearrange("p (c f) -> p c f", f=128))
            # attn @ bv (unnormalized)
            with tc.tile_pool(name="p_bvc", bufs=1) as p_bvc, \
                 tc.tile_pool(name="p_bv", bufs=2, space="PSUM") as p_bvp:
                bv_sb = p_bvc.tile([128, EB, 256], f32r)
                nc.sync.dma_start(bv_sb[:, :, :], bv_in[:, :, :])
                for mb in range(MB):
                    psbv = p_bvp.tile([128, 256], f32, tag="psbv")
                    for fkt in range(EB):
                        nc.tensor.matmul(
                            psbv[:, :],
                            attnT_sb[:, fkt, mb * 128:(mb + 1) * 128],
                            bv_sb[:, fkt, :],
                            start=(fkt == 0), stop=(fkt == EB - 1))
                    nc.vector.tensor_copy(abv_sb[:, mb:mb + 1], psbv[:, 0:1])
            with tc.tile_pool(name="p_wv", bufs=3) as p_wv, \
                 tc.tile_pool(name="p_ps", bufs=2, space="PSUM") as p_pps:
                for eb in range(EB):
                    wv_sb = p_wv.tile([128, EB, 128], f32r, tag="wv")
                    nc.scalar.dma_start(wv_sb[:, :, :], wv_t[eb])
                    ps = p_pps.tile([128, 1024], f32, tag="ps")
                    for fkt in range(EB):
                        lhsT = wv_sb[:, fkt, :]
                        nc.tensor.matmul(ps[:, 0:512], lhsT,
                                         attnT_sb[:, fkt, 0:512],
                                         start=(fkt == 0), stop=False)
                        nc.tensor.matmul(ps[:, 512:1024], lhsT,
                                         attnT_sb[:, fkt, 512:1024],
                                         start=(fkt == 0),
                                         stop=(fkt == EB - 1))
                    nc.scalar.copy(pT_sb[:, eb, :], ps[:, :])
            att_ctx.close()  # free attnT_sb before the out phase

            # ========= Phase out: out = P^T-tiles @ x^T =========
            for mb in range(MB):
                nc.vector.tensor_scalar(
                    out=biasf_sb[:, mb:mb + 1], in0=abv_sb[:, mb:mb + 1],
                    scalar1=rsum_sb[:, mb:mb + 1], scalar2=None, op0=Alu.mult)
            with tc.tile_pool(name="o_xt", bufs=2) as p_xt, \
                 tc.tile_pool(name="o_st", bufs=4) as p_ost, \
                 tc.tile_pool(name="o_ps", bufs=4, space="PSUM") as p_ops:
                for sc in range(8):
                    xt_sb = p_xt.tile([128, EB, 512], f32r, tag="xt")
                    nc.scalar.dma_start(
                        xt_sb[:, :, :],
                        xt[:, sc * 512:(sc + 1) * 512].rearrange(
                            "(e p) s -> p e s", p=128))
                    for mb in range(MB):
                        ps = p_ops.tile([128, 512], f32, tag="ps")
                        for e in range(EB):
                            nc.tensor.matmul(
                                ps[:, :],
                                pT_sb[:, e, mb * 128:(mb + 1) * 128],
                                xt_sb[:, e, :],
                                start=(e == 0), stop=(e == EB - 1))
                        osb = p_ost.tile([128, 512], f32, tag="osb")
                        nc.scalar.activation(
                            osb[:, :], ps[:, :], Act.Identity,
                            bias=biasf_sb[:, mb:mb + 1],
                            scale=rsum_sb[:, mb:mb + 1])
                        nc.sync.dma_start(
                            outt[mb * 128:(mb + 1) * 128,
                                 sc * 512:(sc + 1) * 512],
                            osb[:, :])

    nc.compile()
    return nc


def phase_2a(nc, tc, g_d, at_d, wqt, fb_range, suffix):
    """A^T[fb-rows, m] = sum_e G[e, fb]^T-tiles @ wqt[e, m] -> at_d[fb]."""
    with tc.tile_pool(name=f"a_wq{suffix}", bufs=1) as p_wq, \
         tc.tile_pool(name=f"a_g{suffix}", bufs=2) as p_g, \
         tc.tile_pool(name=f"a_st{suffix}", bufs=2) as p_ast, \
         tc.tile_pool(name=f"a_ps{suffix}", bufs=2, space="PSUM") as p_aps:
        wqt_sb = p_wq.tile([128, EB, EH], f32r, name=f"wqt_sb{suffix}")
        for q in range(4):
            nc.sync.dma_start(
                wqt_sb[:, q * 4:(q + 1) * 4, :],
                wqt[q * 512:(q + 1) * 512, :].rearrange(
                    "(e p) m -> p e m", p=128))
        for fb in fb_range:
            g_sb = p_g.tile([128, EB, 128], f32r, tag="g", name=f"g{suffix}")
            nc.scalar.dma_start(
                g_sb[:, :, :],
                g_d[:, :, fb * 128:(fb + 1) * 128].rearrange(
                    "e p f -> p e f"))
            ps = p_aps.tile([128, 1024], f32, tag="ps", name=f"ps{suffix}")
            for e in range(EB):
                lhsT = g_sb[:, e, :]
                nc.tensor.matmul(ps[:, 0:512], lhsT, wqt_sb[:, e, 0:512],
                                 start=(e == 0), stop=False)
                nc.tensor.matmul(ps[:, 512:1024], lhsT,
                                 wqt_sb[:, e, 512:1024],
                                 start=(e == 0), stop=(e == EB - 1))
            st_ = p_ast.tile([128, 1024], f32r, tag="ast",
                             name=f"ast{suffix}")
            nc.scalar.copy(st_[:, :], ps[:, :])
            nc.sync.dma_start(at_d[fb], st_[:, :])


_NC_CACHE = {}


def _get_nc():
    if "nc" not in _NC_CACHE:
        _NC_CACHE["nc"] = build_kernel()
    return _NC_CACHE["nc"]


def make_in_maps(x, Wq, bq, Wk, bk, Wv, bv):
    sc = np.float32(1.0 / np.sqrt(E))
    x = np.asarray(x, np.float32)
    Wq = np.asarray(Wq, np.float32)
    bq = np.asarray(bq, np.float32)
    Wk = np.asarray(Wk, np.float32)
    bk = np.asarray(bk, np.float32)
    Wv = np.asarray(Wv, np.float32)
    bv = np.asarray(bv, np.float32)

    wkt_s = np.ascontiguousarray(Wk.T)                      # [E, E]
    wv_tiled = np.ascontiguousarray(
        Wv.reshape(EB, 128, EB, 128).transpose(2, 1, 0, 3)  # [eb][fp][fb][e]
    )
    bv_pack = np.ascontiguousarray(
        np.broadcast_to(bv.reshape(EB, 128).T[:, :, None],
                        (128, EB, 256)))                    # [128, EB, 256]

    per_batch = []
    for b in range(B):
        xb = np.ascontiguousarray(x[b])                     # [S, E]
        xtb = np.ascontiguousarray(x[b].T)                  # [E, S]
        r = xb.sum(axis=0, dtype=np.float64).astype(np.float32)  # [E]
        kr = (Wk @ r).astype(np.float32)                    # [E]
        per_batch.append((xb, xtb, r, kr))

    in_maps = []
    for c in range(N_CORES):
        b, h = c // 2, c % 2
        xb, xtb, r, kr = per_batch[b]
        wq_h = Wq[h * EH:(h + 1) * EH, :] * sc              # [EH, E]
        wqt_h = np.ascontiguousarray(wq_h.T)                # [E, EH]
        btq = bq[h * EH:(h + 1) * EH] * sc
        qr = (wq_h @ r).astype(np.float32)
        u = np.ascontiguousarray(
            np.stack([btq, qr + np.float32(S) * btq]))      # [2, EH]
        w = np.ascontiguousarray(np.stack([kr, bk]))        # [2, E]
        in_maps.append({
            "x_se": xb,
            "xt": xtb,
            "wqt": wqt_h,
            "wkt": wkt_s,
            "wv_t": wv_tiled,
            "u_in": u,
            "w_in": w,
            "bv_in": bv_pack,
        })
    return in_maps


def run(in_maps, trace=False, **kwargs):
    nc = _get_nc()
    return run_bass_kernel_spmd(
        nc, in_maps, core_ids=list(range(N_CORES)), trace=trace, **kwargs
    )


def kernel(x, Wq, bq, Wk, bk, Wv, bv):
    in_maps = make_in_maps(x, Wq, bq, Wk, bk, Wv, bv)
    res = run(in_maps, trace=False)
    out = np.empty((B, E, S), dtype=np.float32)
    for c in range(N_CORES):
        b, h = c // 2, c % 2
        out[b, h * EH:(h + 1) * EH, :] = res.results[c]["outt"]
    return out
